# revision 5
# baseline (speedup 1.0000x reference)
"""CPL loss (all-support) Trainium2 kernel.

Math reformulation
------------------
Reference: for each query q, gather S=91 sample queries (90 negatives drawn per
class via a fixed jax PRNG + the query itself), compute cosine similarity of the
20 supports of q's class against the 91 samples, log-softmax over samples, NLL
at the self position, mean, then an extra 1/nq.

Because every sample is itself one of the 1000 queries, all needed cosine
similarities live in the 200x1000 support-x-query Gram matrix ``Ghat``.  With
``Ehat = exp(Ghat)`` the per-(support,query) softmax denominator is

    SumExp[r, q] = sum_{s in samples(q)} Ehat[r, s]  =  (Ehat @ Mask)[r, q]

where Mask[q', q] counts occurrences of query q' in q's sample multiset (host
precomputed - it depends only on the PRNG + labels, not on float data). Only the
20 rows of q's class matter, so per label L we matmul the 20-column slice of
``Ehat^T`` against Mask's columns of label-L queries, producing a (20, 1000)
array. The loss is

    loss = (Sum2 - Sum1) / (nq * K * nq)
    Sum1 = sum_{q,k} Ghat[20*lbl(q)+k, q]          (target logits)
    Sum2 = sum_{q,k} log(SumExp[k, q])             (log denominators)

Sharding: queries are label-sorted and sharded 8 x 125 across cores
(data-parallel over the query axis, support set replicated). Each core computes
its 125 rows of Ghat^T / Ehat^T plus the partial mask-matmul over its 125
contraction rows; one AllReduce of the (20,1000) fp32 partial finishes the
denominators; each core then logs + reduces its own column slice. Host sums the
8 partial (Sum1, Sum2) scalars and applies the constants.
"""

import os
import numpy as np

import concourse.bass as bass
import concourse.mybir as mybir
import concourse.tile as tile
from concourse import bass_utils
from concourse.vector_clock import ScopedClock

N_WAY = 10
Q_PER = 100
K_SHOT = 20
D = 2048
M_NEG = 10
NQ = N_WAY * Q_PER          # 1000
NS = N_WAY * K_SHOT         # 200
S_SAMP = (N_WAY - 1) * M_NEG + 1  # 91
N_CORES = 8
QSH = NQ // N_CORES         # 125
KT = D // 128               # 16
HALF = 500                  # psum column split (<= 512 fp32 per bank)

F32 = mybir.dt.float32

_last_exec_time_ns = None
_last_results = None


def _mk_wait(nc, engine, w):
    wi = mybir.InstEventSemaphore(name=nc.get_next_instruction_name(), engine=engine)
    wi.sync_info = mybir.SyncInfo(on_wait=[w], on_update=[])
    return wi


class _TileContextSplitWaits(tile.TileContext):
    """Workaround for a walrus build that rejects >1 sync-wait per
    instruction: peel extra waits onto standalone single-wait EventSemaphore
    instructions on the same (in-order) engine queue."""

    def _add_instruction(self, inst):
        si = inst.sync_info
        if si is not None and si.on_wait and len(si.on_wait) > 1:
            waits = list(si.on_wait)
            for w in waits[:-1]:
                super()._add_instruction(_mk_wait(self.nc, inst.engine, w))
            si.on_wait = waits[-1:]
        super()._add_instruction(inst)

    def _drain_and_barrier(self, tick_clock, wait_clock):
        nc = self.nc
        drain_inst = nc.sync.drain()
        wait_clock.add_sem_waits(
            drain_inst.ins, ScopedClock({None: tick_clock.global_clock})
        )
        si = drain_inst.ins.sync_info
        waits = list(si.on_wait or [])
        if len(waits) > 1:
            si.on_wait = waits[:1]
            for w in waits[1:]:
                self._add_instruction(_mk_wait(nc, drain_inst.ins.engine, w))

        nc.all_engine_barrier()
        assert self.sems is not None
        popped = nc._tile_sem_poison_stack.pop()
        assert popped is self._sem_poison
        nc.clear_and_free_semaphores(list(self.sems.allocated().values()))
        nc.all_engine_barrier()


def _sample_idx(labels_query: np.ndarray) -> np.ndarray:
    """Replicate the reference's per-query negative sampling exactly."""
    import jax
    import jax.numpy as jnp

    cpu = jax.devices("cpu")[0]
    with jax.default_device(cpu):
        key = jax.random.key(42)
        u = jax.random.uniform(key, (NQ, N_WAY, Q_PER))
        _, topm = jax.lax.top_k(u, M_NEG)
        lbl = jnp.asarray(labels_query).astype(jnp.int32)
        j = jnp.arange(N_WAY - 1)
        other = j[None, :] + (j[None, :] >= lbl[:, None])
        sel = jnp.take_along_axis(topm, other[:, :, None], axis=1)
        neg_idx = (other[:, :, None] * Q_PER + sel).reshape(NQ, -1)
        sample_idx = jnp.concatenate([neg_idx, jnp.arange(NQ)[:, None]], axis=1)
        return np.asarray(sample_idx)


def _tileize_dT(mat_t: np.ndarray, ncols: int) -> np.ndarray:
    """(D, ncols) -> (128, KT*ncols) laid out so SBUF free slice k*ncols:(k+1)*ncols
    is the k-th 128-row chunk of the D-major matrix."""
    return np.ascontiguousarray(
        mat_t.reshape(KT, 128, ncols).transpose(1, 0, 2).reshape(128, KT * ncols)
    )


def _build_program(chunks):
    """Build the SPMD Bass program (identical on all 8 cores)."""
    nc = bass.Bass("TRN2", num_devices=N_CORES)

    qt_d = nc.dram_tensor("qt", [128, KT * QSH], F32, kind="ExternalInput")
    st_d = nc.dram_tensor("st", [128, KT * NS], F32, kind="ExternalInput")
    mask_d = nc.dram_tensor("mask", [QSH, NQ], F32, kind="ExternalInput")
    rt_d = nc.dram_tensor("rt", [QSH, NS], F32, kind="ExternalInput")
    ownw_d = nc.dram_tensor("ownw", [K_SHOT, NQ], F32, kind="ExternalInput")
    out_d = nc.dram_tensor("out", [1, 2], F32, kind="ExternalOutput")

    with _TileContextSplitWaits(nc) as tc:
        with (
            tc.tile_pool(name="sb", bufs=1) as sb,
            tc.tile_pool(name="ps", bufs=1, space="PSUM") as ps,
            tc.tile_pool(name="dram", bufs=1, space="DRAM") as dram,
        ):
            ones_col = sb.tile([128, 1], F32, tag="ones_col")
            nc.gpsimd.memset(ones_col[:], 1.0)
            ones_row = sb.tile([1, 128], F32, tag="ones_row")
            nc.gpsimd.memset(ones_row[:], 1.0)

            qt = sb.tile([128, KT * QSH], F32, tag="qt")
            nc.sync.dma_start(qt[:], qt_d[:, :])
            st = sb.tile([128, KT * NS], F32, tag="st")
            nc.sync.dma_start(st[:], st_d[:, :])
            maskt = sb.tile([QSH, NQ], F32, tag="maskt")
            nc.sync.dma_start(maskt[:], mask_d[:, :])
            rt = sb.tile([QSH, NS], F32, tag="rt")
            nc.sync.dma_start(rt[:], rt_d[:, :])
            ownw = sb.tile([K_SHOT, NQ], F32, tag="ownw")
            nc.sync.dma_start(ownw[:], ownw_d[:, :])

            # ---- norms: sum of squares over D (partition k-tiles) ----
            qt2 = sb.tile([128, KT * QSH], F32, tag="qt2")
            nc.scalar.activation(qt2[:], qt[:], mybir.ActivationFunctionType.Square)
            st2 = sb.tile([128, KT * NS], F32, tag="st2")
            nc.scalar.activation(st2[:], st[:], mybir.ActivationFunctionType.Square)

            qred = sb.tile([128, QSH], F32, tag="qred")
            nc.vector.reduce_sum(
                qred[:],
                qt2[:].rearrange("p (k c) -> p c k", k=KT),
                axis=mybir.AxisListType.X,
            )
            sred = sb.tile([128, NS], F32, tag="sred")
            nc.vector.reduce_sum(
                sred[:],
                st2[:].rearrange("p (k c) -> p c k", k=KT),
                axis=mybir.AxisListType.X,
            )

            ps_qn = ps.tile([1, QSH], F32, tag="psA")
            nc.tensor.matmul(ps_qn[:], ones_col[:], qred[:], start=True, stop=True)
            ps_sn = ps.tile([1, NS], F32, tag="psB")
            nc.tensor.matmul(ps_sn[:], ones_col[:], sred[:], start=True, stop=True)

            # inverse norms: 1/sqrt(ssq)
            qn_sq = sb.tile([1, QSH], F32, tag="qn_sq")
            nc.scalar.activation(qn_sq[:], ps_qn[:], mybir.ActivationFunctionType.Sqrt)
            qn_i = sb.tile([1, QSH], F32, tag="qn_i")
            nc.vector.reciprocal(qn_i[:], qn_sq[:])
            sn_sq = sb.tile([1, NS], F32, tag="sn_sq")
            nc.scalar.activation(sn_sq[:], ps_sn[:], mybir.ActivationFunctionType.Sqrt)
            sn_i = sb.tile([1, NS], F32, tag="sn_i")
            nc.vector.reciprocal(sn_i[:], sn_sq[:])

            # c_q: (1,QSH) -> (QSH,1) via PE (K=1 matmul against [1,1] ones)
            ps_cq = ps.tile([QSH, 1], F32, tag="psA")
            nc.tensor.matmul(
                ps_cq[:], qn_i[:], ones_row[:, 0:1], start=True, stop=True
            )
            c_q = sb.tile([QSH, 1], F32, tag="c_q")
            nc.scalar.copy(c_q[:], ps_cq[:])

            # broadcast sn_i across partitions: (128, NS)
            ps_bc = ps.tile([128, NS], F32, tag="psB")
            nc.tensor.matmul(ps_bc[:], ones_row[:], sn_i[:], start=True, stop=True)
            bc = sb.tile([128, NS], F32, tag="bc")
            nc.scalar.copy(bc[:], ps_bc[:])

            # ---- main Gram matmul: G^T_j = Q_j @ S^T  (QSH x NS) ----
            ps_g = ps.tile([QSH, NS], F32, tag="ps_g")
            for k in range(KT):
                nc.tensor.matmul(
                    ps_g[:],
                    qt[:, k * QSH : (k + 1) * QSH],
                    st[:, k * NS : (k + 1) * NS],
                    start=(k == 0),
                    stop=(k == KT - 1),
                )

            # scale by support inverse norms (free-dim broadcast tile)
            t_scaled = sb.tile([QSH, NS], F32, tag="t_scaled")
            nc.vector.tensor_tensor(
                t_scaled[:], ps_g[:], bc[0:QSH, :], mybir.AluOpType.mult
            )

            # Sum1 partial: sum_k sum_s (t_scaled * rt) * c_q  (per-partition,
            # then a K=QSH ones matmul for the partition reduce)
            junk = sb.tile([QSH, NS], F32, tag="junk")
            v1 = sb.tile([QSH, 1], F32, tag="v1")
            nc.vector.tensor_tensor(
                junk[:], t_scaled[:], rt[:], mybir.AluOpType.mult
            )
            nc.vector.reduce_sum(v1[:], junk[:], axis=mybir.AxisListType.X)
            v2 = sb.tile([QSH, 1], F32, tag="v2")
            nc.vector.tensor_tensor(v2[:], v1[:], c_q[:], mybir.AluOpType.mult)
            ps_s1 = ps.tile([1, 1], F32, tag="psA")
            nc.tensor.matmul(
                ps_s1[:], v2[:], ones_col[0:QSH, :], start=True, stop=True
            )

            # Ehat^T rows for this shard: exp(t_scaled * c_q) fused on ACT
            ehat = sb.tile([QSH, NS], F32, tag="ehat")
            nc.scalar.activation(
                ehat[:], t_scaled[:], mybir.ActivationFunctionType.Exp, scale=c_q[:]
            )

            # ---- per-label mask matmuls -> partial SumExp (K_SHOT x NQ) ----
            ps_seA = ps.tile([K_SHOT, HALF], F32, tag="ps_seA")
            ps_seB = ps.tile([K_SHOT, HALF], F32, tag="ps_seB")
            halves = {0: ps_seA, 1: ps_seB}
            for (lab, coloff, bank, within, take) in chunks:
                nc.tensor.matmul(
                    halves[bank][:, within : within + take],
                    ehat[:, K_SHOT * lab : K_SHOT * (lab + 1)],
                    maskt[:, coloff : coloff + take],
                    start=True,
                    stop=True,
                )

            se_sbuf = sb.tile([K_SHOT, NQ], F32, tag="se_sbuf")
            nc.scalar.copy(se_sbuf[:, 0:HALF], ps_seA[:])
            nc.scalar.copy(se_sbuf[:, HALF:NQ], ps_seB[:])

            # ---- AllReduce the partial SumExp across the 8 cores ----
            ar_in = dram.tile([K_SHOT, NQ], F32, tag="ar_in")
            ar_out = dram.tile([K_SHOT, NQ], F32, tag="ar_out")
            nc.sync.dma_start(ar_in[:], se_sbuf[:])
            nc.gpsimd.collective_compute(
                "AllReduce",
                mybir.AluOpType.add,
                replica_groups=[list(range(N_CORES))],
                ins=[ar_in.opt()],
                outs=[ar_out.opt()],
            )
            se_sum = sb.tile([K_SHOT, NQ], F32, tag="se_sum")
            nc.sync.dma_start(se_sum[:], ar_out[:])

            # Sum2 partial: log, mask to own columns, reduce
            lg = sb.tile([K_SHOT, NQ], F32, tag="lg")
            nc.scalar.activation(lg[:], se_sum[:], mybir.ActivationFunctionType.Ln)
            junk2 = sb.tile([K_SHOT, NQ], F32, tag="junk2")
            v3 = sb.tile([K_SHOT, 1], F32, tag="v3")
            nc.vector.tensor_tensor(
                junk2[:], lg[:], ownw[:], mybir.AluOpType.mult
            )
            nc.vector.reduce_sum(v3[:], junk2[:], axis=mybir.AxisListType.X)
            ps_s2 = ps.tile([1, 1], F32, tag="psB")
            nc.tensor.matmul(
                ps_s2[:], v3[:], ones_col[0:K_SHOT, :], start=True, stop=True
            )

            outt = sb.tile([1, 2], F32, tag="outt")
            nc.scalar.copy(outt[:, 0:1], ps_s1[:])
            nc.scalar.copy(outt[:, 1:2], ps_s2[:])
            nc.sync.dma_start(out_d[:, :], outt[:])

    return nc



def _enable_tracing():
    """Best-effort NTFF profiling under axon: install the missing
    antenv.axon_hooks shim + skip the artifact upload."""
    import sys
    import types

    if "antenv.axon_hooks" not in sys.modules:
        mod = types.ModuleType("antenv.axon_hooks")
        mod._hook = None

        def set_axon_ntff_profile_hook(h):
            mod._hook = h

        def get_axon_ntff_profile_hook():
            return mod._hook

        mod.set_axon_ntff_profile_hook = set_axon_ntff_profile_hook
        mod.get_axon_ntff_profile_hook = get_axon_ntff_profile_hook
        sys.modules["antenv.axon_hooks"] = mod
        try:
            from trn_agent_boot.trn_boot import _ntff_profile_via_ctypes

            mod._hook = _ntff_profile_via_ctypes("/opt/axon/libaxon_pjrt.so")
        except Exception as e:
            print("tracing hook unavailable:", e)
    bass_utils.upload_artifacts = lambda tmpdir: "local://skipped"


def kernel(support_set, queries, labels_query, labels_support):
    global _last_exec_time_ns, _last_results

    support_set = np.ascontiguousarray(np.asarray(support_set, dtype=np.float32))
    queries = np.ascontiguousarray(np.asarray(queries, dtype=np.float32))
    lbl = np.asarray(labels_query).astype(np.int64)

    # ---- host-side index prep (PRNG + labels only; no float math) ----
    sample_idx = _sample_idx(lbl.astype(np.int32))          # (NQ, 91)
    order = np.argsort(lbl, kind="stable")                  # sorted-query order
    pos = np.empty(NQ, dtype=np.int64)
    pos[order] = np.arange(NQ)
    lbl_sorted = lbl[order]

    counts = np.bincount(lbl, minlength=N_WAY)
    offs = np.concatenate([[0], np.cumsum(counts)])

    # psum-bank-safe matmul chunks: (label, col_off, bank, col_within, take)
    chunks = []
    for lab in range(N_WAY):
        off, n = int(offs[lab]), int(counts[lab])
        while n > 0:
            bank = off // HALF
            within = off % HALF
            take = min(n, HALF - within)
            chunks.append((lab, off, bank, within, take))
            off += take
            n -= take

    # mask: counts of sorted-query q' in sorted-column q's sample multiset
    samp_pos = pos[sample_idx[order]]                        # (NQ, 91)
    mask_full = np.zeros((NQ, NQ), dtype=np.float32)
    np.add.at(
        mask_full,
        (samp_pos.ravel(), np.repeat(np.arange(NQ), S_SAMP)),
        1.0,
    )

    # target-row mask (sorted query rows x supports)
    rt_full = np.zeros((NQ, NS), dtype=np.float32)
    rt_cols = lbl_sorted[:, None] * K_SHOT + np.arange(K_SHOT)[None, :]
    rt_full[np.arange(NQ)[:, None], rt_cols] = 1.0

    st_tiled = _tileize_dT(support_set.T, NS)

    in_maps = []
    for j in range(N_CORES):
        sl = slice(j * QSH, (j + 1) * QSH)
        qs = queries[order[sl]]                              # (QSH, D)
        qt_tiled = _tileize_dT(np.ascontiguousarray(qs.T), QSH)
        ownw = np.zeros((K_SHOT, NQ), dtype=np.float32)
        ownw[:, sl] = 1.0
        in_maps.append(
            {
                "qt": qt_tiled,
                "st": st_tiled,
                "mask": np.ascontiguousarray(mask_full[sl, :]),
                "rt": np.ascontiguousarray(rt_full[sl, :]),
                "ownw": ownw,
            }
        )

    nc = _build_program(chunks)
    trace = os.environ.get("KERNEL_TRACE", "0") == "1"
    if trace:
        _enable_tracing()
    res = bass_utils.run_bass_kernel_spmd(
        nc, in_maps, core_ids=list(range(N_CORES)), trace=trace
    )
    _last_exec_time_ns = res.exec_time_ns
    _last_results = res

    parts = np.stack([res.results[j]["out"][0] for j in range(N_CORES)])  # (8, 2)
    sum1 = np.float32(parts[:, 0].sum(dtype=np.float64))
    sum2 = np.float32(parts[:, 1].sum(dtype=np.float64))
    loss = (sum2 - sum1) / np.float32(NQ * K_SHOT) / np.float32(NQ)
    return np.asarray(loss, dtype=np.float32)


# revision 7
# speedup vs baseline: 1.9614x; 1.9614x over previous
"""CPL loss (all-support) Trainium2 kernel - no-collective SPMD design.

Math reformulation
------------------
Reference: for each query q, gather S=91 sample queries (90 negatives drawn per
class via a fixed jax PRNG + the query itself), compute cosine similarity of the
20 supports of q's class against the 91 samples, log-softmax over samples, NLL
at the self position, mean over (q, k), then an extra 1/nq.

Every sample is itself one of the 1000 queries, so all needed cosine
similarities are entries of the support x query Gram matrix ``Ghat``. With
``Ehat = exp(Ghat)`` the per-(support,query) softmax denominator is

    SumExp[r, q] = sum_{s in samples(q)} Ehat[r, s] = (Ehat @ Mask)[r, q]

where Mask[q', q] counts occurrences of query q' in q's sample multiset (host
precomputed - it depends only on the PRNG + labels, not on float data).

    loss = (Sum2 - Sum1) / (nq * K * nq)
    Sum1 = sum_{q,k} Ghat[20*lbl(q)+k, q]          (target logits)
    Sum2 = sum_{q,k} log(SumExp[20*lbl(q)+k, q])   (log denominators)

Sharding (no collectives - measured cost of ANY collective in this runtime is
~65us of barrier/skew/ncfw overhead, far more than the redundant compute it
saves): queries are label-sorted and sharded 8 x 125. A 125-query window of
the sorted order touches few labels (2 for the block-labeled episodic layout),
so core j only needs the Gram slab of its shard's `20*n_lab` support rows
against ALL 1000 queries (sample multisets span every query). Each core
computes its slab, both norm sets, the mask matmul over the full contraction,
and its own (Sum1_j, Sum2_j) partials; the host sums 8 partial pairs and
applies the constants (the unshard/gather step).

Per-core device pipeline (matmul inputs bf16, accumulation fp32):
  1. ssq_q via ACT/DVE squares of the d-major query tiles + a PE ones-matmul
     reduction -> (1,1000); transposed to per-partition chunks by tiny K=1
     matmuls; 1/sqrt via ACT Sqrt + DVE reciprocal. Same for the supports.
  2. slab = S_lab^T-tiles x Q^T-tiles -> psum (nsl x 1000) fp32.
  3. per 128-column chunk: PE transpose -> (128 x nsl), scale by support norms
     (broadcast tile), Exp(scale=query-norm) on ACT -> Ehat^T chunk (bf16);
     masked target-term accumulation on DVE for Sum1.
  4. mask matmul: 8 chunk matmuls accumulate psum (nsl x 125) = SumExp for
     this shard's own 125 query columns.
  5. Ln on ACT, row-ownership mask, reductions -> (Sum1_j, Sum2_j) -> DRAM.
"""

import os
import numpy as np
import ml_dtypes

import concourse.bass as bass
import concourse.mybir as mybir
import concourse.tile as tile
from concourse import bass_utils
from concourse.vector_clock import ScopedClock

N_WAY = 10
Q_PER = 100
K_SHOT = 20
D = 2048
M_NEG = 10
NQ = N_WAY * Q_PER          # 1000
NS = N_WAY * K_SHOT         # 200
S_SAMP = (N_WAY - 1) * M_NEG + 1  # 91
N_CORES = 8
QSH = NQ // N_CORES         # 125
KT = D // 128               # 16
NB = 512                    # psum bank f32 capacity (column split)
NCH = (NQ + 127) // 128     # 8 query chunks
ACT_SQ_TILES = 6            # square tiles 0..5 on ACT, rest on DVE

F32 = mybir.dt.float32
BF16 = mybir.dt.bfloat16
BF16_NP = ml_dtypes.bfloat16

_last_exec_time_ns = None
_last_results = None


def _mk_wait(nc, engine, w):
    wi = mybir.InstEventSemaphore(name=nc.get_next_instruction_name(), engine=engine)
    wi.sync_info = mybir.SyncInfo(on_wait=[w], on_update=[])
    return wi


class _TileContextSplitWaits(tile.TileContext):
    """Workaround for a walrus build that rejects >1 sync-wait per
    instruction: peel extra waits onto standalone single-wait EventSemaphore
    instructions on the same (in-order) engine queue."""

    def _add_instruction(self, inst):
        si = inst.sync_info
        if si is not None and si.on_wait and len(si.on_wait) > 1:
            waits = list(si.on_wait)
            for w in waits[:-1]:
                super()._add_instruction(_mk_wait(self.nc, inst.engine, w))
            si.on_wait = waits[-1:]
        super()._add_instruction(inst)

    def _drain_and_barrier(self, tick_clock, wait_clock):
        nc = self.nc
        drain_inst = nc.sync.drain()
        wait_clock.add_sem_waits(
            drain_inst.ins, ScopedClock({None: tick_clock.global_clock})
        )
        si = drain_inst.ins.sync_info
        waits = list(si.on_wait or [])
        if len(waits) > 1:
            si.on_wait = waits[:1]
            for w in waits[1:]:
                self._add_instruction(_mk_wait(nc, drain_inst.ins.engine, w))

        nc.all_engine_barrier()
        assert self.sems is not None
        popped = nc._tile_sem_poison_stack.pop()
        assert popped is self._sem_poison
        nc.clear_and_free_semaphores(list(self.sems.allocated().values()))
        nc.all_engine_barrier()


def _sample_idx(labels_query: np.ndarray) -> np.ndarray:
    """Replicate the reference's per-query negative sampling exactly."""
    import jax
    import jax.numpy as jnp

    cpu = jax.devices("cpu")[0]
    with jax.default_device(cpu):
        key = jax.random.key(42)
        u = jax.random.uniform(key, (NQ, N_WAY, Q_PER))
        _, topm = jax.lax.top_k(u, M_NEG)
        lbl = jnp.asarray(labels_query).astype(jnp.int32)
        j = jnp.arange(N_WAY - 1)
        other = j[None, :] + (j[None, :] >= lbl[:, None])
        sel = jnp.take_along_axis(topm, other[:, :, None], axis=1)
        neg_idx = (other[:, :, None] * Q_PER + sel).reshape(NQ, -1)
        sample_idx = jnp.concatenate([neg_idx, jnp.arange(NQ)[:, None]], axis=1)
        return np.asarray(sample_idx)


def _tileize_dT(mat_t: np.ndarray, ncols: int, dtype) -> np.ndarray:
    """(D, ncols) -> (128, KT*ncols): free slice k*ncols:(k+1)*ncols is the
    k-th 128-row chunk of the D-major matrix."""
    return np.ascontiguousarray(
        mat_t.reshape(KT, 128, ncols).transpose(1, 0, 2).reshape(128, KT * ncols)
    ).astype(dtype)


def _tileize_rows(mat: np.ndarray, width: int, dtype) -> np.ndarray:
    """(NQ, width) -> (128, NCH*width): free slice c*width:(c+1)*width is rows
    [128c, 128c+128) (zero-padded past NQ)."""
    padded = np.zeros((NCH * 128, width), mat.dtype)
    padded[:NQ] = mat
    return np.ascontiguousarray(
        padded.reshape(NCH, 128, width).transpose(1, 0, 2).reshape(128, NCH * width)
    ).astype(dtype)


def _build_program(n_lab: int):
    """Build the SPMD Bass program (identical on all 8 cores)."""
    nsl = K_SHOT * n_lab  # slab rows (40 for block labels)
    nc = bass.Bass("TRN2", num_devices=N_CORES)

    qt_d = nc.dram_tensor("qt", [128, KT * NQ], BF16, kind="ExternalInput")
    st_d = nc.dram_tensor("st", [128, KT * nsl], BF16, kind="ExternalInput")
    mask_d = nc.dram_tensor("mask", [128, NCH * QSH], BF16, kind="ExternalInput")
    rmask_d = nc.dram_tensor("rmask", [128, NCH * nsl], F32, kind="ExternalInput")
    rowm_d = nc.dram_tensor("rowm", [nsl, QSH], F32, kind="ExternalInput")
    ident_d = nc.dram_tensor("ident", [nsl, nsl], F32, kind="ExternalInput")
    out_d = nc.dram_tensor("out", [1, 2], F32, kind="ExternalOutput")

    QDMA_SPLIT = 4  # qt arrives in pieces so the slab can start early
    KT_PER = KT // QDMA_SPLIT

    with _TileContextSplitWaits(nc) as tc:
        with (
            tc.tile_pool(name="sb", bufs=1) as sb,
            tc.tile_pool(name="sb2", bufs=2) as sb2,
            tc.tile_pool(name="ps", bufs=1, space="PSUM") as ps,
            tc.tile_pool(name="pst", bufs=2, space="PSUM") as pst,
        ):
            ones_col = sb.tile([128, 1], BF16, tag="ones_col")
            nc.gpsimd.memset(ones_col[:], 1.0)
            ones_row = sb.tile([1, 128], F32, tag="ones_row")
            nc.gpsimd.memset(ones_row[:], 1.0)
            ones_col_f = sb.tile([128, 1], F32, tag="ones_col_f")
            nc.gpsimd.memset(ones_col_f[:], 1.0)

            qt = sb.tile([128, KT * NQ], BF16, tag="qt")
            for s in range(QDMA_SPLIT):
                w = KT_PER * NQ
                nc.sync.dma_start(
                    qt[:, s * w : (s + 1) * w], qt_d[:, s * w : (s + 1) * w]
                )
            st = sb.tile([128, KT * nsl], BF16, tag="st")
            nc.sync.dma_start(st[:], st_d[:, :])
            maskt = sb.tile([128, NCH * QSH], BF16, tag="maskt")
            nc.sync.dma_start(maskt[:], mask_d[:, :])
            rmask = sb.tile([128, NCH * nsl], F32, tag="rmask")
            nc.sync.dma_start(rmask[:], rmask_d[:, :])
            rowm = sb.tile([nsl, QSH], F32, tag="rowm")
            nc.sync.dma_start(rowm[:], rowm_d[:, :])
            ident = sb.tile([nsl, nsl], F32, tag="ident")
            nc.sync.dma_start(ident[:], ident_d[:, :])

            # ---- squares of query tiles (split ACT / DVE) ----
            qt2 = sb.tile([128, KT * NQ], BF16, tag="qt2")
            for k in range(KT):
                src = qt[:, k * NQ : (k + 1) * NQ]
                dst = qt2[:, k * NQ : (k + 1) * NQ]
                if k < ACT_SQ_TILES:
                    nc.scalar.activation(dst, src, mybir.ActivationFunctionType.Square)
                else:
                    nc.vector.tensor_tensor(dst, src, src, mybir.AluOpType.mult)

            # ---- main Gram slab (nsl x NQ) fp32 + ssq ones-reduction ----
            ps_slab = ps.tile([nsl, NQ], F32, tag="ps_slab")
            for lo, hi in ((0, NB), (NB, NQ)):
                for k in range(KT):
                    nc.tensor.matmul(
                        ps_slab[:, lo:hi],
                        st[:, k * nsl : (k + 1) * nsl],
                        qt[:, k * NQ + lo : k * NQ + hi],
                        start=(k == 0),
                        stop=(k == KT - 1),
                    )
            ps_ssq = ps.tile([1, NQ], F32, tag="ps_ssq")
            for lo, hi in ((0, NB), (NB, NQ)):
                for k in range(KT):
                    nc.tensor.matmul(
                        ps_ssq[:, lo:hi],
                        ones_col[:],
                        qt2[:, k * NQ + lo : k * NQ + hi],
                        start=(k == 0),
                        stop=(k == KT - 1),
                    )

            # ---- support norms: a_s = 1/sqrt(ssq_s), broadcast (128 x nsl) --
            st2 = sb.tile([128, KT * nsl], BF16, tag="st2")
            nc.scalar.activation(st2[:], st[:], mybir.ActivationFunctionType.Square)
            ps_sn = ps.tile([1, nsl], F32, tag="psnorm")
            for k in range(KT):
                nc.tensor.matmul(
                    ps_sn[:],
                    ones_col[:],
                    st2[:, k * nsl : (k + 1) * nsl],
                    start=(k == 0),
                    stop=(k == KT - 1),
                )
            sn_sq = sb.tile([1, nsl], F32, tag="sn_sq")
            nc.scalar.activation(sn_sq[:], ps_sn[:], mybir.ActivationFunctionType.Sqrt)
            sn_i = sb.tile([1, nsl], F32, tag="sn_i")
            nc.vector.reciprocal(sn_i[:], sn_sq[:])
            ps_abc = ps.tile([128, nsl], F32, tag="psnorm")
            nc.tensor.matmul(ps_abc[:], ones_row[:], sn_i[:], start=True, stop=True)
            a_bc = sb.tile([128, nsl], F32, tag="a_bc")
            nc.scalar.copy(a_bc[:], ps_abc[:])

            # ---- query inverse norms per chunk: (128,1) x NCH ----
            ssq_row = sb.tile([1, NQ], F32, tag="ssq_row")
            nc.vector.tensor_copy(ssq_row[:], ps_ssq[:])
            c_chunks = []
            for c in range(NCH):
                pn = 128 if (c + 1) * 128 <= NQ else NQ - c * 128
                ps_cq = pst.tile([128, 1], F32, tag="scr")
                nc.tensor.matmul(
                    ps_cq[0:pn, :],
                    ssq_row[:, c * 128 : c * 128 + pn],
                    ones_row[0:1, 0:1],
                    start=True,
                    stop=True,
                )
                csq = sb.tile([128, 1], F32, tag=f"csq{c}")
                nc.scalar.activation(
                    csq[0:pn, :], ps_cq[0:pn, :], mybir.ActivationFunctionType.Sqrt
                )
                cch = sb.tile([128, 1], F32, tag=f"cch{c}")
                nc.vector.reciprocal(cch[0:pn, :], csq[0:pn, :])
                c_chunks.append(cch)

            # ---- slab -> sbuf, then per-chunk transpose / scale / exp ----
            gs = sb.tile([nsl, NQ], F32, tag="gs")
            nc.scalar.copy(gs[:, 0:NB], ps_slab[:, 0:NB])
            nc.scalar.copy(gs[:, NB:NQ], ps_slab[:, NB:NQ])

            ehat = sb.tile([128, NCH * nsl], BF16, tag="ehat")
            v_acc = sb.tile([128, 1], F32, tag="v_acc")
            nc.gpsimd.memset(v_acc[:], 0.0)
            for c in range(NCH):
                pn = 128 if (c + 1) * 128 <= NQ else NQ - c * 128
                ps_t = pst.tile([128, nsl], F32, tag="scr")
                nc.tensor.transpose(
                    ps_t[0:pn, :], gs[:, c * 128 : c * 128 + pn], ident[:]
                )
                tmp = sb2.tile([128, nsl], F32, tag="tmp")
                nc.vector.tensor_tensor(
                    tmp[0:pn, :], ps_t[0:pn, :], a_bc[0:pn, :], mybir.AluOpType.mult
                )
                # Ehat^T chunk = exp(tmp * c_q), fused scale on ACT
                nc.scalar.activation(
                    ehat[0:pn, c * nsl : (c + 1) * nsl],
                    tmp[0:pn, :],
                    mybir.ActivationFunctionType.Exp,
                    scale=c_chunks[c][0:pn, :],
                )
                # target-term partial: sum_r (tmp * c_q) * rmask -> v_acc
                ghat_c = sb2.tile([128, nsl], F32, tag="ghat_c")
                nc.vector.tensor_scalar_mul(
                    ghat_c[0:pn, :], tmp[0:pn, :], c_chunks[c][0:pn, :]
                )
                mskd = sb2.tile([128, nsl], F32, tag="mskd")
                nc.vector.tensor_tensor(
                    mskd[0:pn, :],
                    ghat_c[0:pn, :],
                    rmask[0:pn, c * nsl : (c + 1) * nsl],
                    mybir.AluOpType.mult,
                )
                rv = sb2.tile([128, 1], F32, tag="rv")
                nc.vector.reduce_sum(
                    rv[0:pn, :], mskd[0:pn, :], axis=mybir.AxisListType.X
                )
                nc.vector.tensor_tensor(
                    v_acc[0:pn, :], v_acc[0:pn, :], rv[0:pn, :], mybir.AluOpType.add
                )

            # ---- mask matmul: SumExp for this shard's own 125 columns ----
            ps_sum = ps.tile([nsl, QSH], F32, tag="ps_sum")
            for c in range(NCH):
                pn = 128 if (c + 1) * 128 <= NQ else NQ - c * 128
                nc.tensor.matmul(
                    ps_sum[:],
                    ehat[0:pn, c * nsl : (c + 1) * nsl],
                    maskt[0:pn, c * QSH : (c + 1) * QSH],
                    start=(c == 0),
                    stop=(c == NCH - 1),
                )

            # ---- Sum2: log + row-ownership mask + reduce ----
            lgt = sb.tile([nsl, QSH], F32, tag="lgt")
            nc.scalar.activation(lgt[:], ps_sum[:], mybir.ActivationFunctionType.Ln)
            lmskd = sb.tile([nsl, QSH], F32, tag="lmskd")
            nc.vector.tensor_tensor(lmskd[:], lgt[:], rowm[:], mybir.AluOpType.mult)
            v2 = sb.tile([nsl, 1], F32, tag="v2")
            nc.vector.reduce_sum(v2[:], lmskd[:], axis=mybir.AxisListType.X)

            ps_s1 = pst.tile([1, 1], F32, tag="scr")
            nc.tensor.matmul(ps_s1[:], v_acc[:], ones_col_f[:], start=True, stop=True)
            ps_s2 = pst.tile([1, 1], F32, tag="scr")
            nc.tensor.matmul(
                ps_s2[:], v2[:], ones_col_f[0:nsl, :], start=True, stop=True
            )

            outt = sb.tile([1, 2], F32, tag="outt")
            nc.scalar.copy(outt[:, 0:1], ps_s1[:])
            nc.scalar.copy(outt[:, 1:2], ps_s2[:])
            nc.sync.dma_start(out_d[:, :], outt[:])

    return nc


def kernel(support_set, queries, labels_query, labels_support):
    global _last_exec_time_ns, _last_results

    support_set = np.ascontiguousarray(np.asarray(support_set, dtype=np.float32))
    queries = np.ascontiguousarray(np.asarray(queries, dtype=np.float32))
    lbl = np.asarray(labels_query).astype(np.int64)

    # ---- host-side index prep (PRNG + labels only; no float math) ----
    sample_idx = _sample_idx(lbl.astype(np.int32))          # (NQ, 91)
    order = np.argsort(lbl, kind="stable")                  # sorted-query order
    pos = np.empty(NQ, dtype=np.int64)
    pos[order] = np.arange(NQ)
    lbl_sorted = lbl[order]

    # per-core label sets, padded to a common size for SPMD uniformity
    core_labs = []
    for j in range(N_CORES):
        labs = sorted(set(lbl_sorted[j * QSH : (j + 1) * QSH].tolist()))
        core_labs.append(labs)
    n_lab = max(len(l) for l in core_labs)
    for labs in core_labs:
        while len(labs) < n_lab:
            labs.append(labs[0])
    nsl = K_SHOT * n_lab

    # full sample-count matrix in sorted coordinates
    samp_pos = pos[sample_idx[order]]                        # (NQ, 91)
    mask_full = np.zeros((NQ, NQ), dtype=np.float32)
    np.add.at(
        mask_full,
        (samp_pos.ravel(), np.repeat(np.arange(NQ), S_SAMP)),
        1.0,
    )

    queries_sorted_T = np.ascontiguousarray(queries[order].T)  # (D, NQ)
    qt_tiled = _tileize_dT(queries_sorted_T, NQ, BF16_NP)

    in_maps = []
    for j in range(N_CORES):
        sl = slice(j * QSH, (j + 1) * QSH)
        labs = core_labs[j]
        sup_rows = np.concatenate(
            [np.arange(L * K_SHOT, (L + 1) * K_SHOT) for L in labs]
        )
        st_j = support_set[sup_rows]                         # (nsl, D)
        # slab-local base row of each label (first occurrence; pads excluded)
        row_of = {}
        for i, L in enumerate(labs):
            if L not in row_of:
                row_of[L] = i * K_SHOT

        # rmask: (q'_sorted, slab_row) ones at own-shard target entries
        rmask_full = np.zeros((NQ, nsl), dtype=np.float32)
        qs = np.arange(j * QSH, (j + 1) * QSH)
        base = np.array([row_of[L] for L in lbl_sorted[sl]])
        rmask_full[qs[:, None], base[:, None] + np.arange(K_SHOT)[None, :]] = 1.0

        # rowm: (slab_row, own_col) ones at the label rows of each column
        rowm = np.zeros((nsl, QSH), dtype=np.float32)
        rows2 = base[:, None] + np.arange(K_SHOT)[None, :]   # (QSH, 20)
        cols2 = np.broadcast_to(np.arange(QSH)[:, None], rows2.shape)
        rowm[rows2.ravel(), cols2.ravel()] = 1.0

        in_maps.append(
            {
                "qt": qt_tiled,
                "st": _tileize_dT(np.ascontiguousarray(st_j.T), nsl, BF16_NP),
                "mask": _tileize_rows(mask_full[:, sl], QSH, BF16_NP),
                "rmask": _tileize_rows(rmask_full, nsl, np.float32),
                "rowm": rowm,
                "ident": np.eye(nsl, dtype=np.float32),
            }
        )

    nc = _build_program(n_lab)
    trace = os.environ.get("KERNEL_TRACE", "0") == "1"
    if trace:
        _enable_tracing()
    res = bass_utils.run_bass_kernel_spmd(
        nc, in_maps, core_ids=list(range(N_CORES)), trace=trace
    )
    _last_exec_time_ns = res.exec_time_ns
    _last_results = res

    parts = np.stack([res.results[j]["out"][0] for j in range(N_CORES)])  # (8, 2)
    sum1 = np.float32(parts[:, 0].sum(dtype=np.float64))
    sum2 = np.float32(parts[:, 1].sum(dtype=np.float64))
    loss = (sum2 - sum1) / np.float32(NQ * K_SHOT) / np.float32(NQ)
    return np.asarray(loss, dtype=np.float32)


def _enable_tracing():
    """Best-effort NTFF profiling under axon: install the missing
    antenv.axon_hooks shim + skip the artifact upload."""
    import sys
    import types

    if "antenv.axon_hooks" not in sys.modules:
        mod = types.ModuleType("antenv.axon_hooks")
        mod._hook = None

        def set_axon_ntff_profile_hook(h):
            mod._hook = h

        def get_axon_ntff_profile_hook():
            return mod._hook

        mod.set_axon_ntff_profile_hook = set_axon_ntff_profile_hook
        mod.get_axon_ntff_profile_hook = get_axon_ntff_profile_hook
        sys.modules["antenv.axon_hooks"] = mod
        try:
            from trn_agent_boot.trn_boot import _ntff_profile_via_ctypes

            mod._hook = _ntff_profile_via_ctypes("/opt/axon/libaxon_pjrt.so")
        except Exception as e:
            print("tracing hook unavailable:", e)
    bass_utils.upload_artifacts = lambda tmpdir: "local://skipped"
